# revision 12
# baseline (speedup 1.0000x reference)
"""PPO loss_fn kernel for Trainium2, 8 NeuronCores, data-parallel over envs.

Sharding: B=2048 envs split across 8 cores (256 envs each). Each core runs
the full pipeline (obs normalize, policy/value MLPs, distribution math,
time-reversed GAE scan, loss partial sums) on its env slice; the host
combines the 8 partial sums into the final scalar.

Device layout is feature-major: columns are tokens ordered b-major with
T+1=129 slots per env (tok = b*129 + t). Policy/distribution quantities at
t=128 are computed but excluded from all reductions.

log_det_j is identical for target and behaviour log-probs (same action), so
it cancels in rho = exp(t_lp - b_lp); only sq/ln terms are reduced, via a
single fp32 matmul against constant +/- coefficient columns.
"""
import sys
sys.path.insert(0, '/opt/trn_rl_repo')

import numpy as np
import ml_dtypes

import concourse.bass as bass
import concourse.bacc as bacc
import concourse.mybir as mybir
from concourse.tile import TileContext
from concourse.bass_utils import run_bass_kernel_spmd

F32 = mybir.dt.float32
BF16 = mybir.dt.bfloat16
ALU = mybir.AluOpType
AF = mybir.ActivationFunctionType

# problem constants
T, B, OBS, ACT, H = 128, 2048, 128, 32, 256
NCORES = 8
BL = B // NCORES              # 256 envs per core
TP1 = T + 1                   # 129
NTV = BL * TP1                # 33024 value tokens per core
HALF = 16512                  # NTV // 2 (128 envs worth of columns)

GAMMA = 0.97
LAMBDA = 0.95
EPS_CLIP = 0.2
LOG2PI = float(np.log(2.0 * np.pi))
LOG2 = float(np.log(2.0))
ENT_CONST = 0.5 + 0.5 * LOG2PI + 2.0 * LOG2   # per (token, act-dim) constant

CHUNKS = [(i * 512, 512) for i in range(64)] + [(32768, 256)]

_CACHE = {}


def _build_program():
    nc = bacc.Bacc("TRN2", debug=False, num_devices=NCORES)

    # ---- DRAM I/O ----
    obsT = nc.dram_tensor("obsT", [128, NTV], F32, kind="ExternalInput")
    qT = nc.dram_tensor("qT", [128, NTV], F32, kind="ExternalInput")
    rewT = nc.dram_tensor("rewT", [BL, T], F32, kind="ExternalInput")
    doneT = nc.dram_tensor("doneT", [BL, T], F32, kind="ExternalInput")
    biasP = nc.dram_tensor("biasP", [128, 18], F32, kind="ExternalInput")
    onesP = nc.dram_tensor("onesP", [128, 4], F32, kind="ExternalInput")
    w_names = ["pw1T", "pw2T0", "pw2T1", "vw1T", "vw2T0", "vw2T1"]
    wdr = {n: nc.dram_tensor(n, [128, 256], BF16, kind="ExternalInput")
           for n in w_names}
    pw3T = nc.dram_tensor("pw3T", [256, 64], BF16, kind="ExternalInput")
    vw3T = nc.dram_tensor("vw3T", [256, 1], BF16, kind="ExternalInput")
    out = nc.dram_tensor("out", [1, 3], F32, kind="ExternalOutput")

    # DRAM scratch
    lsvD = nc.dram_tensor("lsvD", [65, NTV], F32)   # loc 0:32, sraw 32:64, val 64
    redAD = nc.dram_tensor("redAD", [2, NTV], F32)  # dlp row 0, ent row 1

    with TileContext(nc) as tc:
        with tc.tile_pool(name="wpool", bufs=1) as wp:
            # persistent weights / constants
            wsb = {n: wp.tile([128, 256], BF16, tag=n, name=n) for n in w_names}
            for n in w_names:
                nc.sync.dma_start(wsb[n][:, :], wdr[n][:, :])
            pw3 = wp.tile([128, 2, 64], BF16, tag="pw3")
            nc.sync.dma_start(pw3[:, :, :], pw3T.rearrange("(k p) m -> p k m", p=128))
            vw3 = wp.tile([128, 2, 1], BF16, tag="vw3")
            nc.sync.dma_start(vw3[:, :, :], vw3T.rearrange("(k p) m -> p k m", p=128))
            bias = wp.tile([128, 18], F32, tag="bias")
            nc.sync.dma_start(bias[:, :], biasP[:, :])
            ones = wp.tile([128, 4], F32, tag="ones")
            nc.sync.dma_start(ones[:, :], onesP[:, :])
            onecol = wp.tile([128, 1], F32, tag="onecol")
            nc.gpsimd.memset(onecol[:, :], 1.0)
            st = wp.tile([128, 8], F32, tag="st")

            # normalization stats: var = clip(rv/(ns+1)); inv_sd; nbias
            nc.vector.tensor_scalar_add(st[:, 0:1], bias[:, 13:14], 1.0)
            nc.vector.reciprocal(st[:, 1:2], st[:, 0:1])
            nc.vector.tensor_mul(st[:, 2:3], bias[:, 12:13], st[:, 1:2])
            nc.vector.tensor_scalar(st[:, 3:4], st[:, 2:3], 1e-6, 1e6,
                                    ALU.max, ALU.min)
            nc.scalar.activation(st[:, 4:5], st[:, 3:4], AF.Sqrt,
                                 bias=bias[:, 14:15])
            nc.vector.reciprocal(st[:, 5:6], st[:, 4:5])
            nc.vector.scalar_tensor_tensor(st[:, 6:7], bias[:, 11:12], -1.0,
                                           st[:, 5:6], ALU.mult, ALU.mult)
            inv_sd = st[:, 5:6]
            nbias = st[:, 6:7]

            # ---- Phase 1: normalize + MLPs (silu table set) ----
            with tc.tile_pool(name="p1", bufs=3) as p1, \
                 tc.tile_pool(name="ps_mm", bufs=4, space="PSUM") as psm, \
                 tc.tile_pool(name="ps_p3", bufs=2, space="PSUM") as psp3:
                for (s, W) in CHUNKS:
                    obsr = p1.tile([128, 512], F32, tag="obsr")
                    nc.sync.dma_start(obsr[:, :W], obsT[:, s:s + W])
                    obsn = p1.tile([128, 512], BF16, tag="obsn")
                    nc.scalar.activation(obsn[:, :W], obsr[:, :W], AF.Identity,
                                         bias=nbias, scale=inv_sd)
                    nc.vector.tensor_scalar(obsn[:, :W], obsn[:, :W], -5.0, 5.0,
                                            ALU.max, ALU.min)

                    hs = {}
                    for br, w1, w2a, w2b, b1c, b2c in (
                            ("p", "pw1T", "pw2T0", "pw2T1", 0, 2),
                            ("v", "vw1T", "vw2T0", "vw2T1", 4, 6)):
                        ph1a = psm.tile([128, 512], F32, tag="mm")
                        ph1b = psm.tile([128, 512], F32, tag="mm")
                        nc.tensor.matmul(ph1a[:, :W], wsb[w1][:, 0:128],
                                         obsn[:, :W], start=True, stop=True)
                        nc.tensor.matmul(ph1b[:, :W], wsb[w1][:, 128:256],
                                         obsn[:, :W], start=True, stop=True)
                        h1a = p1.tile([128, 512], BF16, tag=br + "h1a")
                        h1b = p1.tile([128, 512], BF16, tag=br + "h1b")
                        nc.scalar.activation(h1a[:, :W], ph1a[:, :W], AF.Silu,
                                             bias=bias[:, b1c:b1c + 1])
                        nc.scalar.activation(h1b[:, :W], ph1b[:, :W], AF.Silu,
                                             bias=bias[:, b1c + 1:b1c + 2])
                        ph2a = psm.tile([128, 512], F32, tag="mm")
                        ph2b = psm.tile([128, 512], F32, tag="mm")
                        nc.tensor.matmul(ph2a[:, :W], wsb[w2a][:, 0:128],
                                         h1a[:, :W], start=True, stop=False)
                        nc.tensor.matmul(ph2a[:, :W], wsb[w2b][:, 0:128],
                                         h1b[:, :W], start=False, stop=True)
                        nc.tensor.matmul(ph2b[:, :W], wsb[w2a][:, 128:256],
                                         h1a[:, :W], start=True, stop=False)
                        nc.tensor.matmul(ph2b[:, :W], wsb[w2b][:, 128:256],
                                         h1b[:, :W], start=False, stop=True)
                        h2a = p1.tile([128, 512], BF16, tag=br + "h2a")
                        h2b = p1.tile([128, 512], BF16, tag=br + "h2b")
                        nc.scalar.activation(h2a[:, :W], ph2a[:, :W], AF.Silu,
                                             bias=bias[:, b2c:b2c + 1])
                        nc.scalar.activation(h2b[:, :W], ph2b[:, :W], AF.Silu,
                                             bias=bias[:, b2c + 1:b2c + 2])
                        hs[br] = (h2a, h2b)

                    p3 = psp3.tile([65, 512], F32, tag="p3")
                    nc.tensor.matmul(p3[0:64, :W], pw3[:, 0, :], hs["p"][0][:, :W],
                                     start=True, stop=False)
                    nc.tensor.matmul(p3[0:64, :W], pw3[:, 1, :], hs["p"][1][:, :W],
                                     start=False, stop=True)
                    nc.tensor.matmul(p3[64:65, :W], vw3[:, 0, :], hs["v"][0][:, :W],
                                     start=True, stop=False)
                    nc.tensor.matmul(p3[64:65, :W], vw3[:, 1, :], hs["v"][1][:, :W],
                                     start=False, stop=True)
                    lsv = p1.tile([65, 512], F32, tag="lsv")
                    nc.scalar.activation(lsv[:, :W], p3[:, :W], AF.Identity,
                                         bias=bias[0:65, 14:15])
                    nc.sync.dma_start(lsvD[:, s:s + W], lsv[:, :W])

            # ---- Phase 2: distribution math (natural_log_exp table set) ----
            with tc.tile_pool(name="p2", bufs=3) as p2, \
                 tc.tile_pool(name="ps_ra", bufs=2, space="PSUM") as psra:
                for (s, W) in CHUNKS:
                    # operands of 2-input SBUF ops must share base partition
                    # (walrus NCC_IBIR297), so loc/b_loc are DMA'd into slots
                    # window-aligned with their partners.
                    M = p2.tile([96, 512], F32, tag="M")
                    nc.sync.dma_start(M[0:32, :W], lsvD[32:64, s:s + W])   # sraw_p
                    nc.sync.dma_start(M[32:64, :W], qT[96:128, s:s + W])   # bsraw
                    nc.sync.dma_start(M[64:96, :W], qT[0:32, s:s + W])     # a
                    M2 = p2.tile([32, 512], F32, tag="M2")
                    nc.sync.dma_start(M2[0:32, :W], qT[32:64, s:s + W])    # eps
                    M3 = p2.tile([96, 512], F32, tag="M3")
                    nc.sync.dma_start(M3[64:96, :W], lsvD[0:32, s:s + W])  # loc_raw
                    M4 = p2.tile([96, 512], F32, tag="M4")
                    nc.sync.dma_start(M4[64:96, :W], qT[64:96, s:s + W])   # b_loc
                    M5 = p2.tile([32, 512], F32, tag="M5")
                    nc.sync.dma_start(M5[0:32, :W], lsvD[0:32, s:s + W])   # loc_raw

                    # softplus(x) = ln(exp(x) + 1)  (no softplus HW table)
                    X = p2.tile([64, 512], F32, tag="X")
                    nc.scalar.activation(X[:, :W], M[0:64, :W], AF.Exp,
                                         bias=bias[0:64, 8:9])
                    SP = p2.tile([64, 512], F32, tag="SP")
                    nc.scalar.activation(SP[:, :W], X[:, :W], AF.Ln,
                                         bias=bias[0:64, 15:16])
                    s2 = p2.tile([64, 512], F32, tag="s2")
                    nc.vector.tensor_scalar_add(s2[:, :W], SP[:, :W], 0.001)
                    R = p2.tile([64, 512], F32, tag="R")
                    nc.vector.reciprocal_approx_fast(R[:, :W], s2[:, :W])

                    FL = p2.tile([128, 512], F32, tag="FL")
                    # ln_s = ln(softplus + 0.001) for p and b
                    nc.scalar.activation(FL[64:128, :W], SP[:, :W], AF.Ln,
                                         bias=bias[0:64, 16:17])

                    Z = p2.tile([64, 512], F32, tag="Z")
                    # z_p = (a - pb3lo) - loc_raw
                    nc.vector.scalar_tensor_tensor(
                        Z[0:32, :W], M[64:96, :W], bias[64:96, 9:10],
                        M3[64:96, :W], ALU.subtract, ALU.subtract)
                    # z_b = a - b_loc
                    nc.vector.tensor_sub(Z[32:64, :W], M[64:96, :W],
                                         M4[64:96, :W])
                    U = p2.tile([64, 512], F32, tag="U")
                    nc.vector.tensor_mul(U[:, :W], Z[:, :W], R[:, :W])
                    nc.vector.tensor_mul(FL[0:64, :W], U[:, :W], U[:, :W])

                    DD = p2.tile([32, 512], F32, tag="DD")
                    nc.vector.tensor_mul(DD[:, :W], s2[0:32, :W], M2[0:32, :W])
                    D = p2.tile([96, 512], F32, tag="D")
                    # dist = (s_p*eps + pb3lo) + loc_raw, clamped for exp safety
                    nc.vector.scalar_tensor_tensor(
                        D[0:32, :W], DD[:, :W], bias[0:32, 9:10],
                        M5[0:32, :W], ALU.add, ALU.add)
                    nc.vector.tensor_scalar(D[0:32, :W], D[0:32, :W],
                                            -43.0, 43.0, ALU.max, ALU.min)
                    nc.scalar.activation(D[32:64, :W], D[0:32, :W], AF.Exp,
                                         bias=bias[0:32, 14:15], scale=-2.0)
                    nc.scalar.activation(D[64:96, :W], D[32:64, :W], AF.Ln,
                                         bias=bias[32:64, 15:16])

                    redA = psra.tile([2, 512], F32, tag="redA")
                    nc.tensor.matmul(redA[:, :W], ones[:, 0:2], FL[:, :W],
                                     start=True, stop=False)
                    nc.tensor.matmul(redA[:, :W], ones[0:96, 2:4], D[:, :W],
                                     start=False, stop=True)
                    redAs = p2.tile([2, 512], F32, tag="redAs")
                    nc.vector.tensor_copy(redAs[:, :W], redA[:, :W])
                    nc.sync.dma_start(redAD[:, s:s + W], redAs[:, :W])

            # ---- Final: GAE + loss partial sums ----
            with tc.tile_pool(name="fin", bufs=2) as fp, \
                 tc.tile_pool(name="ps_f", bufs=1, space="PSUM") as psf:
                acc = fp.tile([128, 6], F32, tag="acc")
                for h in (0, 1):
                    sl = slice(h * HALF, (h + 1) * HALF)
                    vals = fp.tile([128, TP1], F32, tag="vals")
                    nc.sync.dma_start(
                        vals[:, :], lsvD[64:65, sl].rearrange(
                            "o (b t) -> (o b) t", b=128, t=TP1))
                    vals2 = fp.tile([128, TP1], F32, tag="vals2")
                    nc.vector.tensor_scalar(vals2[:, :], vals[:, :],
                                            bias[:, 10:11], None, ALU.add)
                    dlpA = fp.tile([128, TP1], F32, tag="dlpA")
                    nc.sync.dma_start(
                        dlpA[:, :], redAD[0:1, sl].rearrange(
                            "o (b t) -> (o b) t", b=128, t=TP1))
                    entA = fp.tile([128, TP1], F32, tag="entA")
                    nc.sync.dma_start(
                        entA[:, :], redAD[1:2, sl].rearrange(
                            "o (b t) -> (o b) t", b=128, t=TP1))
                    rew = fp.tile([128, T], F32, tag="rew")
                    nc.sync.dma_start(rew[:, :],
                                      rewT[h * 128:(h + 1) * 128, :])
                    don = fp.tile([128, T], F32, tag="don")
                    nc.sync.dma_start(don[:, :],
                                      doneT[h * 128:(h + 1) * 128, :])

                    rho = fp.tile([128, T], F32, tag="rho")
                    nc.scalar.activation(rho[:, :], dlpA[:, 0:T], AF.Exp,
                                         bias=bias[:, 14:15])

                    c1 = fp.tile([128, T], F32, tag="c1")
                    nc.vector.tensor_scalar(c1[:, :], don[:, :], -GAMMA, GAMMA,
                                            ALU.mult, ALU.add)
                    mask = fp.tile([128, T], F32, tag="mask")
                    nc.vector.tensor_scalar(mask[:, :], don[:, :], -1.0, 1.0,
                                            ALU.mult, ALU.add)
                    t1 = fp.tile([128, T], F32, tag="t1")
                    nc.vector.tensor_mul(t1[:, :], c1[:, :], vals2[:, 1:TP1])
                    nc.vector.tensor_add(t1[:, :], t1[:, :], rew[:, :])
                    nc.vector.tensor_sub(t1[:, :], t1[:, :], vals2[:, 0:T])
                    deltas = fp.tile([128, T], F32, tag="deltas")
                    nc.vector.tensor_mul(deltas[:, :], t1[:, :], mask[:, :])
                    factor = fp.tile([128, T], F32, tag="factor")
                    nc.vector.tensor_scalar_mul(factor[:, :], c1[:, :], LAMBDA)

                    vsmv = fp.tile([128, T], F32, tag="vsmv")
                    nc.vector.tensor_tensor_scan(
                        vsmv[:, ::-1], factor[:, ::-1], deltas[:, ::-1],
                        0.0, ALU.mult, ALU.add)

                    vsb = fp.tile([128, TP1], F32, tag="vsb")
                    nc.vector.tensor_add(vsb[:, 0:T], vsmv[:, :],
                                         vals2[:, 0:T])
                    nc.vector.tensor_copy(vsb[:, T:TP1], vals2[:, T:TP1])
                    b1 = fp.tile([128, T], F32, tag="b1")
                    nc.vector.tensor_mul(b1[:, :], c1[:, :], vsb[:, 1:TP1])
                    nc.vector.tensor_add(b1[:, :], b1[:, :], rew[:, :])
                    nc.vector.tensor_sub(b1[:, :], b1[:, :], vals2[:, 0:T])
                    adv = fp.tile([128, T], F32, tag="adv")
                    nc.vector.tensor_mul(adv[:, :], b1[:, :], mask[:, :])

                    s1 = fp.tile([128, T], F32, tag="s1")
                    nc.vector.tensor_mul(s1[:, :], rho[:, :], adv[:, :])
                    rc = fp.tile([128, T], F32, tag="rc")
                    nc.vector.tensor_scalar(rc[:, :], rho[:, :],
                                            1.0 - EPS_CLIP, 1.0 + EPS_CLIP,
                                            ALU.max, ALU.min)
                    nc.vector.tensor_mul(rc[:, :], rc[:, :], adv[:, :])
                    smin = fp.tile([128, T], F32, tag="smin")
                    nc.vector.tensor_tensor(smin[:, :], s1[:, :], rc[:, :],
                                            ALU.min)
                    vsq = fp.tile([128, T], F32, tag="vsq")
                    nc.vector.tensor_mul(vsq[:, :], vsmv[:, :], vsmv[:, :])

                    nc.vector.tensor_reduce(acc[:, 3 * h + 0:3 * h + 1],
                                            smin[:, :], mybir.AxisListType.X,
                                            ALU.add)
                    nc.vector.tensor_reduce(acc[:, 3 * h + 1:3 * h + 2],
                                            vsq[:, :], mybir.AxisListType.X,
                                            ALU.add)
                    nc.vector.tensor_reduce(acc[:, 3 * h + 2:3 * h + 3],
                                            entA[:, 0:T], mybir.AxisListType.X,
                                            ALU.add)

                accs = fp.tile([128, 3], F32, tag="accs")
                nc.vector.tensor_add(accs[:, :], acc[:, 0:3], acc[:, 3:6])
                redF = psf.tile([1, 3], F32, tag="redF")
                nc.tensor.matmul(redF[:, :], onecol[:, :], accs[:, :],
                                 start=True, stop=True)
                redFs = fp.tile([1, 3], F32, tag="redFs")
                nc.vector.tensor_copy(redFs[:, :], redF[:, :])
                nc.sync.dma_start(out[:, :], redFs[:, :])

    nc.compile()
    return nc


def _prep_inputs(inputs):
    """Host-side shard + layout transforms (no math beyond parameter packing)."""
    obs = np.asarray(inputs["observation"], np.float32)     # [129,2048,128]
    rew = np.asarray(inputs["reward"], np.float32)          # [128,2048]
    done = np.asarray(inputs["done"], np.float32)
    logits = np.asarray(inputs["logits"], np.float32)       # [128,2048,64]
    action = np.asarray(inputs["action"], np.float32)       # [128,2048,32]
    eps = np.asarray(inputs["eps_noise"], np.float32)

    def padT(x):  # [T,BL,32] -> [32, BL, 129] -> [32, NTV]
        y = np.zeros((x.shape[2], x.shape[1], TP1), np.float32)
        y[:, :, :T] = x.transpose(2, 1, 0)
        return y.reshape(x.shape[2], -1)

    bf = ml_dtypes.bfloat16
    wmap = {
        "pw1T": np.ascontiguousarray(np.asarray(inputs["pW1"]).T, dtype=bf),
        "pw2T0": np.ascontiguousarray(np.asarray(inputs["pW2"]).T[0:128], dtype=bf),
        "pw2T1": np.ascontiguousarray(np.asarray(inputs["pW2"]).T[128:256], dtype=bf),
        "vw1T": np.ascontiguousarray(np.asarray(inputs["vW1"]).T, dtype=bf),
        "vw2T0": np.ascontiguousarray(np.asarray(inputs["vW2"]).T[0:128], dtype=bf),
        "vw2T1": np.ascontiguousarray(np.asarray(inputs["vW2"]).T[128:256], dtype=bf),
        "pw3T": np.ascontiguousarray(np.asarray(inputs["pW3"]).T, dtype=bf),
        "vw3T": np.ascontiguousarray(np.asarray(inputs["vW3"]).T, dtype=bf),
    }

    pB1 = np.asarray(inputs["pB1"], np.float32)
    pB2 = np.asarray(inputs["pB2"], np.float32)
    pB3 = np.asarray(inputs["pB3"], np.float32)
    vB1 = np.asarray(inputs["vB1"], np.float32)
    vB2 = np.asarray(inputs["vB2"], np.float32)
    vB3 = np.asarray(inputs["vB3"], np.float32)
    biasP = np.zeros((128, 18), np.float32)
    biasP[:, 0] = pB1[0:128]; biasP[:, 1] = pB1[128:256]
    biasP[:, 2] = pB2[0:128]; biasP[:, 3] = pB2[128:256]
    biasP[:, 4] = vB1[0:128]; biasP[:, 5] = vB1[128:256]
    biasP[:, 6] = vB2[0:128]; biasP[:, 7] = vB2[128:256]
    biasP[0:32, 8] = pB3[32:64]                      # exp bias (scale-raw rows)
    biasP[:, 9] = np.tile(pB3[0:32], 4)              # loc bias, replicated
    biasP[:, 10] = vB3[0]
    biasP[:, 11] = np.asarray(inputs["running_mean"], np.float32)
    biasP[:, 12] = np.asarray(inputs["running_variance"], np.float32)
    biasP[:, 13] = np.float32(np.asarray(inputs["num_steps"]))
    # col 14 stays zero (zero-bias column)
    biasP[:, 15] = 1.0
    biasP[:, 16] = 0.001

    onesP = np.zeros((128, 4), np.float32)
    # MM1 over FL rows [sq_p, sq_b, ln_p, ln_b] -> col0 dlp, col1 ent
    onesP[0:32, 0] = -0.5; onesP[32:64, 0] = 0.5
    onesP[64:96, 0] = -1.0; onesP[96:128, 0] = 1.0
    onesP[64:96, 1] = 1.0
    # MM2 over D rows [dist, e2(junk), spm2d] -> col1 ent
    onesP[0:32, 3] = -2.0; onesP[64:96, 3] = -2.0

    per_core = []
    for c in range(NCORES):
        sl = slice(c * BL, (c + 1) * BL)
        obsTc = np.ascontiguousarray(
            obs[:, sl, :].transpose(2, 1, 0).reshape(128, NTV))
        qTc = np.concatenate([
            padT(action[:, sl, :]),
            padT(eps[:, sl, :]),
            padT(logits[:, sl, 0:32]),
            padT(logits[:, sl, 32:64]),
        ], axis=0)
        m = {
            "obsT": obsTc,
            "qT": np.ascontiguousarray(qTc),
            "rewT": np.ascontiguousarray(rew[:, sl].T),
            "doneT": np.ascontiguousarray(done[:, sl].T),
            "biasP": biasP,
            "onesP": onesP,
        }
        m.update(wmap)
        per_core.append(m)
    return per_core


def get_program():
    if "nc" not in _CACHE:
        _CACHE["nc"] = _build_program()
    return _CACHE["nc"]


def combine(results):
    s0 = sum(float(r["out"][0, 0]) for r in results)
    s1 = sum(float(r["out"][0, 1]) for r in results)
    s2 = sum(float(r["out"][0, 2]) for r in results)
    N = float(T * B)
    # rho = exp(t_lp - b_lp) overflows fp32 whenever a behaviour scale is
    # tiny; the reference then produces inf * 0 = NaN inside the policy-loss
    # mean. Mirror that: a non-finite (or overflowed-on-device) partial sum
    # means the true fp32 loss is NaN.
    if not np.isfinite(s0) or abs(s0) > 1e30:
        return np.float32(np.nan)
    policy_loss = -s0 / N
    v_loss = 0.25 * s1 / N
    entropy_mean = s2 / N + ACT * ENT_CONST
    return np.float32(policy_loss + v_loss - 0.01 * entropy_mean)


def kernel(**inputs) -> np.ndarray:
    nc = get_program()
    per_core = _prep_inputs(inputs)
    res = run_bass_kernel_spmd(nc, per_core, list(range(NCORES)))
    _CACHE["last_results"] = res.results
    return combine(res.results)
